# revision 11
# baseline (speedup 1.0000x reference)
"""Trainium2 Bass kernel for nn_Encoder: 3-layer attention encoder.

Full inputs -> full outputs (out [B,S,D], attns [L,B,H,S,S]).

Sharding (8 cores): core c = b*4 + g handles batch b = c//4 and the head
pair {2g, 2g+1}, for all 3 layers.  Each core writes its 6 attention
slices [S,S] (the memory-bound 805MB output is split 8 ways) and a
partial final-layer output; partial layer activations are summed with an
on-device AllReduce across each batch group of 4 cores, and the final
output partials are summed on the host.

Math notes:
 - softmax without max-subtraction (scores bounded, |s| < ~16; exact in fp32)
 - 1/sqrt(dk) folded into Wq on the host
 - attention probabilities computed in [q,k] layout (exp + per-partition
   normalize); the attn @ v contraction instead uses a second scores
   matmul in [k,q] layout + exp, avoiding on-chip transposes of the big
   matrix; the normalization 1/den is applied per-partition after the
   ctx @ wfc matmul in [s,d] orientation.
"""

import math
import sys

import numpy as np

for _p in ("/opt/trn_rl_repo",):
    if _p not in sys.path:
        sys.path.insert(0, _p)

from concourse import bacc, bass, tile  # noqa: E402
import concourse.mybir as mybir  # noqa: E402
from concourse.bass_utils import run_bass_kernel_spmd  # noqa: E402

B, S, D, H, DK, DV, L = 2, 2048, 256, 8, 32, 32, 3
NCORES = 8
F32 = mybir.dt.float32
AF = mybir.ActivationFunctionType


def _positional_encoding(seq_len, d_model):
    pos = np.arange(seq_len, dtype=np.float32)[:, None]
    div = np.exp(
        np.arange(0, d_model, 2, dtype=np.float32)
        * np.float32(-np.log(10000.0) / d_model)
    )
    pe = np.zeros((seq_len, d_model), np.float32)
    pe[:, 0::2] = np.sin(pos * div)
    pe[:, 1::2] = np.cos(pos * div)
    return pe


def build_nc(S_=S):
    """Build the per-core Bass module (SPMD: same program, per-core data)."""
    NQ = S_ // 128  # number of 128-row tiles along a sequence axis
    AC = min(1024, S_)  # free-dim per ACT instruction / psum scores tile
    n_ac = S_ // AC  # ACT chunks per [128, S_] tile (1 or 2)
    GROUP = min(4, NQ)  # q_tiles per denominator batch

    nc = bacc.Bacc(
        "TRN2", target_bir_lowering=False, debug=False, num_devices=NCORES
    )

    x0T_d = nc.dram_tensor("x0T", [D, S_], F32, kind="ExternalInput")
    wq_d = nc.dram_tensor("wq", [L, D, 64], F32, kind="ExternalInput")
    wk_d = nc.dram_tensor("wk", [L, D, 64], F32, kind="ExternalInput")
    wv_d = nc.dram_tensor("wv", [L, D, 64], F32, kind="ExternalInput")
    wfc_d = nc.dram_tensor("wfc", [L, 64, D], F32, kind="ExternalInput")
    ident_d = nc.dram_tensor("ident", [128, 128], F32, kind="ExternalInput")

    attn_o = nc.dram_tensor("attn_out", [L, 2, S_, S_], F32, kind="ExternalOutput")
    outp_o = nc.dram_tensor("out_partial", [S_, D], F32, kind="ExternalOutput")

    with tile.TileContext(nc) as tc:
        with (
            tc.tile_pool(name="const", bufs=1) as constp,
            tc.tile_pool(name="xt", bufs=2) as xtp,
            tc.tile_pool(name="xtpart", bufs=1) as xtpartp,
            tc.tile_pool(name="qk", bufs=2) as qkp,
            tc.tile_pool(name="vp", bufs=2) as vp,
            tc.tile_pool(name="exp", bufs=6) as expp,
            tc.tile_pool(name="expT", bufs=3) as expTp,
            tc.tile_pool(name="stats", bufs=4) as statsp,
            tc.tile_pool(name="ctxsb", bufs=2) as ctxsbp,
            tc.tile_pool(name="xn", bufs=1) as xnp,
            tc.tile_pool(name="tmp", bufs=2) as tmpp,
            tc.tile_pool(name="sc", bufs=2, space="PSUM") as scp,
            tc.tile_pool(name="ctxps", bufs=1, space="PSUM") as ctxpsp,
            tc.tile_pool(name="ccdram", bufs=2, space="DRAM") as ccp,
        ):
            # ---- load constants ----
            ident = constp.tile([128, 128], F32, tag="ident")
            nc.sync.dma_start(ident[:, :], ident_d[:, :])
            wq_sb = constp.tile([128, L * 128], F32, tag="wq_sb")
            wk_sb = constp.tile([128, L * 128], F32, tag="wk_sb")
            wv_sb = constp.tile([128, L * 128], F32, tag="wv_sb")
            wfc_sb = constp.tile([32, L * 2 * D], F32, tag="wfc_sb")
            for l in range(L):
                for wsb, wd in ((wq_sb, wq_d), (wk_sb, wk_d), (wv_sb, wv_d)):
                    nc.sync.dma_start(
                        wsb[:, l * 128 : (l + 1) * 128].rearrange(
                            "p (c f) -> p c f", c=2
                        ),
                        wd[l, :, :].rearrange("(c p) f -> p c f", p=128),
                    )
                for h in range(2):
                    nc.sync.dma_start(
                        wfc_sb[:, (2 * l + h) * D : (2 * l + h + 1) * D],
                        wfc_d[l, 32 * h : 32 * h + 32, :],
                    )

            # ---- layer 0 input (pre-transposed on host) ----
            xT = xtp.tile([128, 2 * S_], F32, tag="xT")
            nc.sync.dma_start(
                xT[:, :].rearrange("p (c f) -> p c f", c=2),
                x0T_d[:, :].rearrange("(c p) f -> p c f", p=128),
            )

            for l in range(L):
                # ---- qkv projections (both heads jointly: 64 cols) ----
                qT = qkp.tile([64, S_], F32, tag="qT")
                kT = qkp.tile([64, S_], F32, tag="kT")
                MM = min(512, S_)
                for tsb, wsb in ((qT, wq_sb), (kT, wk_sb)):
                    for c0 in range(0, S_, MM):
                        ps = scp.tile([64, MM], F32, tag="sc")
                        for dc in range(2):
                            nc.tensor.matmul(
                                ps[:, :],
                                lhsT=wsb[:, l * 128 + dc * 64 : l * 128 + dc * 64 + 64],
                                rhs=xT[:, dc * S_ + c0 : dc * S_ + c0 + MM],
                                start=(dc == 0),
                                stop=(dc == 1),
                            )
                        nc.vector.tensor_copy(tsb[:, c0 : c0 + MM], ps[:, :])
                v_sb = vp.tile([128, NQ * 64], F32, tag="v_sb")
                for t in range(NQ):
                    ps = scp.tile([128, 64], F32, tag="sc")
                    for dc in range(2):
                        nc.tensor.matmul(
                            ps[:, :],
                            lhsT=xT[:, dc * S_ + t * 128 : dc * S_ + (t + 1) * 128],
                            rhs=wv_sb[:, l * 128 + dc * 64 : l * 128 + dc * 64 + 64],
                            start=(dc == 0),
                            stop=(dc == 1),
                        )
                    nc.vector.tensor_copy(v_sb[:, t * 64 : (t + 1) * 64], ps[:, :])

                ctx_sbs = []
                recips = []
                for h in range(2):
                    dk_sl = slice(32 * h, 32 * h + 32)
                    den_a = statsp.tile([128, NQ], F32, tag="den_a")
                    den_b = statsp.tile([128, NQ], F32, tag="den_b")
                    den_s = statsp.tile([128, NQ], F32, tag="den_s")
                    recip = statsp.tile([128, NQ], F32, tag="recip")
                    ctx_ps = ctxpsp.tile([32, S_], F32, tag="ctx")
                    group = []
                    for i in range(NQ):
                        # ---- pass B: scoresT[k_tile i, :] -> exp -> ctx accum
                        expT_t = expTp.tile([128, S_], F32, tag="expT")
                        for c in range(n_ac):
                            psB = scp.tile([128, AC], F32, tag="sc")
                            for s0 in range(0, AC, 512):
                                nn = min(512, AC - s0)
                                nc.tensor.matmul(
                                    psB[:, s0 : s0 + nn],
                                    lhsT=kT[dk_sl, i * 128 : (i + 1) * 128],
                                    rhs=qT[dk_sl, c * AC + s0 : c * AC + s0 + nn],
                                    start=True,
                                    stop=True,
                                )
                            nc.scalar.activation(
                                expT_t[:, c * AC : (c + 1) * AC],
                                psB[:, :],
                                AF.Exp,
                            )
                        for cc in range(0, S_, MM):
                            nc.tensor.matmul(
                                ctx_ps[:, cc : cc + MM],
                                lhsT=v_sb[:, i * 64 + 32 * h : i * 64 + 32 * h + 32],
                                rhs=expT_t[:, cc : cc + MM],
                                start=(i == 0),
                                stop=(i == NQ - 1),
                            )
                        # ---- pass A: scores[q_tile i, :] -> exp(+den) ----
                        exp_t = expp.tile([128, S_], F32, tag="exp")
                        for c in range(n_ac):
                            psA = scp.tile([128, AC], F32, tag="sc")
                            for s0 in range(0, AC, 512):
                                nn = min(512, AC - s0)
                                nc.tensor.matmul(
                                    psA[:, s0 : s0 + nn],
                                    lhsT=qT[dk_sl, i * 128 : (i + 1) * 128],
                                    rhs=kT[dk_sl, c * AC + s0 : c * AC + s0 + nn],
                                    start=True,
                                    stop=True,
                                )
                            nc.scalar.activation(
                                exp_t[:, c * AC : (c + 1) * AC],
                                psA[:, :],
                                AF.Exp,
                                accum_out=(den_a if c == 0 else den_b)[:, i : i + 1],
                            )
                        group.append((i, exp_t))
                        # ---- denominator batch + normalize + store ----
                        if len(group) == GROUP or i == NQ - 1:
                            g0 = group[0][0]
                            gn = len(group)
                            if n_ac == 2:
                                nc.vector.tensor_add(
                                    den_s[:, g0 : g0 + gn],
                                    den_a[:, g0 : g0 + gn],
                                    den_b[:, g0 : g0 + gn],
                                )
                                dsrc = den_s
                            else:
                                dsrc = den_a
                            nc.vector.reciprocal(
                                recip[:, g0 : g0 + gn], dsrc[:, g0 : g0 + gn]
                            )
                            for j, t in group:
                                nc.vector.tensor_scalar_mul(
                                    t[:, :], t[:, :], recip[:, j : j + 1]
                                )
                                nc.sync.dma_start(
                                    attn_o[l, h, j * 128 : (j + 1) * 128, :],
                                    t[:, :],
                                )
                            group = []
                    ctx_sb = ctxsbp.tile([32, S_], F32, tag="ctxsb")
                    nc.vector.tensor_copy(ctx_sb[:, :], ctx_ps[:, :])
                    ctx_sbs.append(ctx_sb)
                    recips.append(recip)

                # ---- x_next[s, d] = sum_h recip_h * (ctx_uh @ wfc_h) ----
                xn = xnp.tile([128, NQ * D], F32, tag="xn")
                for st in range(NQ):
                    for h in range(2):
                        psX = scp.tile([128, D], F32, tag="sc")
                        nc.tensor.matmul(
                            psX[:, :],
                            lhsT=ctx_sbs[h][:, st * 128 : (st + 1) * 128],
                            rhs=wfc_sb[:, (2 * l + h) * D : (2 * l + h + 1) * D],
                            start=True,
                            stop=True,
                        )
                        if h == 0:
                            nc.vector.tensor_scalar_mul(
                                xn[:, st * D : (st + 1) * D],
                                psX[:, :],
                                recips[0][:, st : st + 1],
                            )
                        else:
                            xtmp = tmpp.tile([128, D], F32, tag="xtmp")
                            nc.vector.tensor_scalar_mul(
                                xtmp[:, :], psX[:, :], recips[1][:, st : st + 1]
                            )
                            nc.vector.tensor_add(
                                xn[:, st * D : (st + 1) * D],
                                xn[:, st * D : (st + 1) * D],
                                xtmp[:, :],
                            )

                if l < L - 1:
                    # transpose partial x [s,d] -> [d,s], AllReduce over the
                    # 4-core batch group, reload as next layer's xT
                    xtpart = xtpartp.tile([128, 2 * S_], F32, tag="xtpart")
                    for dc in range(2):
                        for st in range(NQ):
                            psT = scp.tile([128, 128], F32, tag="sc")
                            nc.tensor.transpose(
                                psT[:, :],
                                xn[:, st * D + dc * 128 : st * D + dc * 128 + 128],
                                ident[:, :],
                            )
                            nc.vector.tensor_copy(
                                xtpart[:, dc * S_ + st * 128 : dc * S_ + (st + 1) * 128],
                                psT[:, :],
                            )
                    cc_in = ccp.tile([D, S_], F32, tag="cc_in")
                    cc_out = ccp.tile([D, S_], F32, tag="cc_out")
                    nc.sync.dma_start(
                        cc_in[:, :].rearrange("(c p) f -> p c f", p=128),
                        xtpart[:, :].rearrange("p (c f) -> p c f", c=2),
                    )
                    nc.gpsimd.collective_compute(
                        "AllReduce",
                        mybir.AluOpType.add,
                        replica_groups=[[0, 1, 2, 3], [4, 5, 6, 7]],
                        ins=[cc_in[:, :]],
                        outs=[cc_out[:, :]],
                    )
                    xT = xtp.tile([128, 2 * S_], F32, tag="xT")
                    nc.sync.dma_start(
                        xT[:, :].rearrange("p (c f) -> p c f", c=2),
                        cc_out[:, :].rearrange("(c p) f -> p c f", p=128),
                    )
                else:
                    nc.sync.dma_start(
                        outp_o[:, :].rearrange("(t p) d -> p t d", p=128),
                        xn[:, :].rearrange("p (t d) -> p t d", t=NQ),
                    )
    if not nc.is_finalized():
        nc.finalize()
    return nc


_NC_CACHE = {}


def _get_nc(S_=S):
    if S_ not in _NC_CACHE:
        _NC_CACHE[S_] = build_nc(S_)
    return _NC_CACHE[S_]


def make_in_maps(seq_inputs, Wq, Wk, Wv, Wfc):
    x0 = seq_inputs.astype(np.float32) + _positional_encoding(S, D)[None]
    x0T = np.ascontiguousarray(x0.transpose(0, 2, 1))  # [B, D, S]
    wq_s = (Wq.astype(np.float32) / np.float32(math.sqrt(DK))).astype(np.float32)
    ident = np.eye(128, dtype=np.float32)
    in_maps = []
    for c in range(NCORES):
        b, g = c // 4, c % 4
        cs = slice(64 * g, 64 * g + 64)  # head columns for heads {2g, 2g+1}
        in_maps.append(
            {
                "x0T": x0T[b],
                "wq": np.ascontiguousarray(wq_s[:, :, cs]),
                "wk": np.ascontiguousarray(Wk.astype(np.float32)[:, :, cs]),
                "wv": np.ascontiguousarray(Wv.astype(np.float32)[:, :, cs]),
                "wfc": np.ascontiguousarray(Wfc.astype(np.float32)[:, cs, :]),
                "ident": ident,
            }
        )
    return in_maps


def assemble(results):
    attns = np.empty((L, B, H, S, S), np.float32)
    out = np.zeros((B, S, D), np.float32)
    for c in range(NCORES):
        b, g = c // 4, c % 4
        r = results[c]
        attns[:, b, 2 * g : 2 * g + 2] = r["attn_out"]
        out[b] += r["out_partial"]
    return out, attns


def kernel(seq_inputs, Wq, Wk, Wv, Wfc, **run_kwargs):
    nc = _get_nc(S)
    in_maps = make_in_maps(seq_inputs, Wq, Wk, Wv, Wfc)
    res = run_bass_kernel_spmd(nc, in_maps, core_ids=list(range(NCORES)), **run_kwargs)
    out, attns = assemble(res.results)
    kernel.last_result = res
    return out, attns


kernel.last_result = None
